# revision 56
# baseline (speedup 1.0000x reference)
"""Sliding-window (chunked) multi-head attention for Trainium2, 8-core SPMD.

Problem: B=1, S=8192, E=512, H=8 heads, Dh=64, window=1024 (half=512).
Reference math per window i (size 1024): keys span [i-512, i+1536).

Sharding: core c owns query window [1024c, 1024c+1024); it receives
x^T for the halo'd key range [1024c-512, 1024c+1536) (zero-padded at
the sequence edges) and computes q/k/v projections locally, windowed
softmax(q k^T / 8) v, and the output projection.  All compute layouts
are transposed ([E, seq]) so every matmul contracts over partitions;
the softmax denominator comes from a ones-augmented v (65th column).

Bias algebra: bv folds into bo_eff on the host (attn rows sum to 1);
bk adds a per-query constant to every logit in a softmax row, which
cancels, so it is dropped; bq is applied during the q-projection
evacuation; bo is accumulated by a PE rank-1 update (bo x ones).

Loop structure: query-half (qc) outer, head inner.  qc=0's output
projection and y writeback overlap qc=1's attention, so only the last
query half sits in the kernel tail.  Per (head, qc): 16 key-tile
iterations of score matmul -> exp (alternating Act table exp / DVE
custom cubic^4 exp) -> AV accumulate.  The denominator reciprocal is
broadcast across partitions with gpsimd partition_broadcast (no DRAM
round-trip) and applied on Pool; the last head of each qc multiplies
on DVE straight from PSUM to shorten the chain into the out-proj.

Outputs are y^T shards [512, 1024] per core; the host transposes and
concatenates.
"""

import numpy as np
import ml_dtypes

import concourse.bass as bass
import concourse.tile as tile
from concourse import bacc, mybir
from concourse import bass_utils
from concourse.bass import ts

# ---- problem constants (hardcoded per contract) ----
S = 8192
E = 512
H = 8
DH = 64
NCORES = 8
SQ = 1024          # queries per core
SK = 2048          # halo'd keys per core
HALF = 512
SCALE = 0.125      # 1/sqrt(64)

F32 = mybir.dt.float32
BF16 = mybir.dt.bfloat16
FP16 = mybir.dt.float16

# ---- custom DVE op: exp(u/8) ~= (1 + c1 u + c2 u^2 + c3 u^3)^4 ----
# Fitted (Lawson minimax) on |u/8| <= 1.6; max rel err 7.2e-4.
_EC1 = 0.03126080224663743
_EC2 = 0.000493647595612354
_EC3 = 5.0261583805949835e-06


def _register_exp_op():
    from concourse import dve_ops as dops
    from concourse.dve_spec import Spec, Src0, One, C0, C1, C2, sq, lower
    from concourse.dve_uop import DveOpSpec

    name = "EXP4_ANT"
    for op in dops.OPS:
        if op.name == name:
            return op
    body = sq(sq(((C2 * Src0 + C1) * Src0 + C0) * Src0 + One))
    spec = Spec(body=body)
    shas = {}
    for ver in ("v3", "v4"):
        uops = lower(spec, ver=ver)
        shas[ver] = DveOpSpec(name=name, opcode=0, uops=uops, rd1_en=False).sha(ver)
    op = dops.DveOp(name, spec, subdim=False, uops_sha=shas)
    dops.OPS.append(op)
    dops.CUSTOM_DVE_SPECS[name] = spec
    dops._SUB_OPCODE_FOR_NAME[name] = dops._CUSTOM_DVE_ROW_BASE + len(dops.OPS) - 1
    assert max(dops._SUB_OPCODE_FOR_NAME.values()) < 0x20
    return op


def _build():
    """Build + compile the per-core Bass program (SPMD: same NEFF, 8 cores)."""
    exp_op = _register_exp_op()

    nc = bacc.Bacc("TRN2", target_bir_lowering=False, debug=False)

    xT_d = nc.dram_tensor("xT", [E, SK], FP16, kind="ExternalInput")
    W_d = {
        n: nc.dram_tensor(n, [128, E // 128, E], FP16, kind="ExternalInput")
        for n in ("Wq", "Wk", "Wv", "Wo")
    }
    bq_d = nc.dram_tensor("bq", [E], F32, kind="ExternalInput")
    bo16_d = nc.dram_tensor("bo16", [E], FP16, kind="ExternalInput")
    mask_d = nc.dram_tensor("mask8", [128, SK // 128, H], FP16, kind="ExternalInput")
    yT_d = nc.dram_tensor("yT", [E, SQ], F32, kind="ExternalOutput")

    KT = 4           # E // 128 contraction tiles
    NKT = SK // 128  # 16 key tiles

    with tile.TileContext(nc) as tc:
        with (
            nc.allow_low_precision(reason="fp16 attention kernel"),
            tc.tile_pool(name="singles", bufs=1) as singles,
            tc.tile_pool(name="exps", bufs=8) as exps,
            tc.tile_pool(name="avus", bufs=3) as avus,
            tc.tile_pool(name="sums", bufs=2) as sums_p,
            tc.tile_pool(name="recips", bufs=2) as recips,
            tc.tile_pool(name="bcs", bufs=2) as bcs,
            tc.tile_pool(name="ystage", bufs=4) as ystage,
        ):
            # ---- load everything.  HBM serializes transfers (each 0.5MB
            # chunk saturates the 358GB/s link), so emission order ~= arrival
            # order.  Wq halves + x centers gate the first q-proj matmuls;
            # k/v tiles that need only center columns are processed first so
            # the halo edges can arrive later.
            W_sb = {}
            for n, d in W_d.items():
                W_sb[n] = singles.tile([128, KT, E], FP16, tag=f"w_{n}", name=f"w_{n}")
            xT_sb = singles.tile([128, KT, SK], FP16)
            nc.sync.dma_start(out=W_sb["Wq"], in_=W_d["Wq"].ap())
            engs = (nc.scalar, nc.gpsimd, nc.sync, nc.scalar)
            for ke in range(KT):
                engs[ke % 3].dma_start(
                    out=xT_sb[:, ke, HALF:HALF + SQ],
                    in_=xT_d[ts(ke, 128), HALF:HALF + SQ])
            nc.gpsimd.dma_start(out=W_sb["Wk"], in_=W_d["Wk"].ap())
            nc.scalar.dma_start(out=W_sb["Wv"], in_=W_d["Wv"].ap())
            for ke in range(KT):
                engs[(ke + 1) % 3].dma_start(
                    out=xT_sb[:, ke, 0:HALF], in_=xT_d[ts(ke, 128), 0:HALF])
            for ke in range(KT):
                engs[(ke + 2) % 3].dma_start(
                    out=xT_sb[:, ke, HALF + SQ:], in_=xT_d[ts(ke, 128), HALF + SQ:])
            bq_sb = singles.tile([128, KT], F32, tag="bq")
            nc.gpsimd.dma_start(out=bq_sb, in_=bq_d.ap().rearrange("(t p) -> p t", p=128))

            # v with ones column (from mask: 0 for padded keys); only needed
            # by the first AV matmul ~45us in
            v_sb = singles.tile([128, NKT, H, DH + 1], FP16, tag="v")
            nc.scalar.dma_start(out=v_sb[:, :, :, DH], in_=mask_d.ap())
            nc.sync.dma_start(out=W_sb["Wo"], in_=W_d["Wo"].ap())
            # bo as a single fp16 row for the PE rank-1 bias accumulate
            boT_sb = singles.tile([1, E], FP16, tag="boT")
            nc.gpsimd.dma_start(out=boT_sb, in_=bo16_d.ap().rearrange("(o e) -> o e", o=1))
            ones_sb = singles.tile([1, 512], FP16, tag="ones")
            nc.vector.memset(ones_sb, 1.0)

            qT_sb = singles.tile([128, KT, SQ], FP16, tag="qT")
            kT_sb = singles.tile([128, KT, SK], FP16, tag="kT")
            outT_sb = singles.tile([128, KT, SQ], FP16, tag="outT")

            # ---- q/k/v projections (kc/st orders follow x-chunk arrival) ----
            with tc.tile_pool(name="pproj", bufs=8, space="PSUM") as pproj:
                for th in range(KT):
                    for qc in range(2):
                        ps = pproj.tile([128, 512], F32, tag="pp")
                        for ke in (0, 1, 2, 3):
                            nc.tensor.matmul(
                                ps,
                                W_sb["Wq"][:, ke, ts(th, 128)],
                                xT_sb[:, ke, HALF + qc * 512:HALF + (qc + 1) * 512],
                                start=(ke == 0), stop=(ke == KT - 1),
                            )
                        # fold bq during evacuation (per-partition scalar add)
                        nc.vector.tensor_scalar_add(
                            out=qT_sb[:, th, ts(qc, 512)], in0=ps,
                            scalar1=bq_sb[:, th:th + 1],
                        )
                    for kc in range(4):
                        ps = pproj.tile([128, 512], F32, tag="pp")
                        for ke in range(KT):
                            nc.tensor.matmul(
                                ps,
                                W_sb["Wk"][:, ke, ts(th, 128)],
                                xT_sb[:, ke, ts(kc, 512)],
                                start=(ke == 0), stop=(ke == KT - 1),
                            )
                        # bk cancels in softmax; plain evacuation on Act
                        nc.scalar.activation(
                            out=kT_sb[:, th, ts(kc, 512)], in_=ps,
                            func=mybir.ActivationFunctionType.Copy,
                        )
                for st in range(NKT):
                    ps = pproj.tile([128, 512], F32, tag="pp")
                    for ke in range(KT):
                        nc.tensor.matmul(
                            ps,
                            xT_sb[:, ke, ts(st, 128)],
                            W_sb["Wv"][:, ke, :],
                            start=(ke == 0), stop=(ke == KT - 1),
                        )
                    nc.scalar.activation(
                        out=v_sb[:, st, :, 0:DH],
                        in_=ps.rearrange("p (h d) -> p h d", h=H),
                        func=mybir.ActivationFunctionType.Copy,
                    )

            # ---- windowed attention + output projection, query-half outer ----
            with (
                tc.tile_pool(name="pscore", bufs=5, space="PSUM") as pscore,
                tc.tile_pool(name="pav", bufs=2, space="PSUM") as pav,
                tc.tile_pool(name="py", bufs=1, space="PSUM") as py,
            ):
                for qc in range(2):
                    for h in range(H):
                        th = h // 2
                        r0 = 64 * (h % 2)
                        av_ps = pav.tile([DH + 1, 512], F32, tag="av")
                        for kt in range(NKT):
                            s_ps = pscore.tile([128, 512], F32, tag="s")
                            nc.tensor.matmul(
                                s_ps,
                                kT_sb[r0:r0 + 64, th, ts(kt, 128)],
                                qT_sb[r0:r0 + 64, th, ts(qc, 512)],
                                start=True, stop=True,
                            )
                            e_sb = exps.tile([128, 512], FP16, tag="e")
                            if kt % 2 == 0:
                                # custom DVE cubic^4 exp (coeffs fold in SCALE)
                                nc.vector._custom_dve(
                                    exp_op, out=e_sb, in0=s_ps,
                                    s0=_EC1, s1=_EC2, imm2=_EC3,
                                )
                            else:
                                nc.scalar.activation(
                                    out=e_sb, in_=s_ps,
                                    func=mybir.ActivationFunctionType.Exp,
                                    scale=SCALE,
                                )
                            nc.tensor.matmul(
                                av_ps,
                                v_sb[:, kt, h, :],
                                e_sb,
                                start=(kt == 0), stop=(kt == NKT - 1),
                            )
                        # normalize: denominator lives in row DH of av_ps.
                        # For h<7: single Act copy evacuates the accumulator
                        # (fast PSUM slot release), sums row sliced on DVE
                        # (custom-DVE recip can't read nonzero partition
                        # offsets), Pool broadcasts + multiplies.  Last head:
                        # DVE multiplies straight from PSUM to shorten the
                        # chain into this qc's out-projection.
                        if h == H - 1:
                            sums_st = sums_p.tile([1, 512], F32, tag="st")
                            nc.scalar.activation(
                                out=sums_st, in_=av_ps[DH:DH + 1, :],
                                func=mybir.ActivationFunctionType.Copy)
                            recip_f = recips.tile([1, 512], F32, tag="rf")
                            nc.vector.reciprocal_approx_fast(out=recip_f, in_=sums_st)
                            bc_sb = bcs.tile([DH, 512], F32, tag="bc")
                            nc.gpsimd.partition_broadcast(bc_sb, recip_f, channels=DH)
                            nc.vector.tensor_mul(
                                out=outT_sb[r0:r0 + 64, th, ts(qc, 512)],
                                in0=av_ps[0:DH, :],
                                in1=bc_sb,
                            )
                        else:
                            avu = avus.tile([DH + 1, 512], F32, tag="avu")
                            nc.scalar.activation(
                                out=avu, in_=av_ps,
                                func=mybir.ActivationFunctionType.Copy)
                            sums_st = sums_p.tile([1, 512], F32, tag="st")
                            nc.vector.tensor_copy(out=sums_st, in_=avu[DH:DH + 1, :])
                            recip_f = recips.tile([1, 512], F32, tag="rf")
                            nc.vector.reciprocal_approx_fast(out=recip_f, in_=sums_st)
                            bc_sb = bcs.tile([DH, 512], F32, tag="bc")
                            nc.gpsimd.partition_broadcast(bc_sb, recip_f, channels=DH)
                            nc.gpsimd.tensor_mul(
                                out=outT_sb[r0:r0 + 64, th, ts(qc, 512)],
                                in0=avu[0:DH, :],
                                in1=bc_sb,
                            )
                    # out-projection for this query half (bias folded in as
                    # a PE rank-1 update); shares the pav pool's bank slots
                    for m in range(KT):
                        ps = py.tile([128, 512], F32, tag="py")
                        for ke in range(KT):
                            nc.tensor.matmul(
                                ps,
                                W_sb["Wo"][:, ke, ts(m, 128)],
                                outT_sb[:, ke, ts(qc, 512)],
                                start=(ke == 0), stop=False,
                            )
                        nc.tensor.matmul(
                            ps, boT_sb[:, ts(m, 128)], ones_sb,
                            start=False, stop=True,
                        )
                        yst = ystage.tile([128, 512], F32, tag="y")
                        nc.scalar.activation(
                            out=yst, in_=ps,
                            func=mybir.ActivationFunctionType.Copy)
                        eng = (nc.sync, nc.scalar)[m % 2]
                        eng.dma_start(out=yT_d[ts(m, 128), ts(qc, 512)], in_=yst)

    nc.compile()
    return nc


_NC_CACHE = []


def _get_nc():
    if not _NC_CACHE:
        _NC_CACHE.append(_build())
    return _NC_CACHE[0]


def _prep_inputs(x, Wq, bq, Wk, bk, Wv, bv, Wo, bo):
    x = np.asarray(x, np.float32)
    xT_full = np.ascontiguousarray(x[0].T)  # [E, S]
    bo_eff = (np.asarray(bo, np.float64)
              + np.asarray(bv, np.float64) @ np.asarray(Wo, np.float64)).astype(np.float32)
    def wprep(W):
        Wb = np.asarray(W, np.float32).astype(np.float16)
        return np.ascontiguousarray(Wb.reshape(4, 128, E).transpose(1, 0, 2))

    shared = {
        "Wq": wprep(Wq),
        "Wk": wprep(Wk),
        "Wv": wprep(Wv),
        "Wo": wprep(Wo),
        "bq": np.asarray(bq, np.float32),
        "bo16": bo_eff.astype(np.float16),
    }
    in_maps = []
    for c in range(NCORES):
        g0 = 1024 * c - HALF
        xT_halo = np.zeros((E, SK), np.float32)
        lo, hi = max(0, g0), min(S, g0 + SK)
        xT_halo[:, lo - g0:hi - g0] = xT_full[:, lo:hi]
        mask = np.zeros((SK, H), np.float32)
        mask[lo - g0:hi - g0, :] = 1.0
        mask = np.ascontiguousarray(mask.reshape(SK // 128, 128, H).transpose(1, 0, 2))
        m = dict(shared)
        m["xT"] = xT_halo.astype(np.float16)
        m["mask8"] = mask.astype(np.float16)
        in_maps.append(m)
    return in_maps


def run(inputs: dict, trace: bool = False):
    nc = _get_nc()
    in_maps = _prep_inputs(**inputs)
    res = bass_utils.run_bass_kernel_spmd(
        nc, in_maps, core_ids=list(range(NCORES)), trace=trace
    )
    y = np.concatenate([r["yT"].T for r in res.results], axis=0)[None]
    return np.ascontiguousarray(y.astype(np.float32)), res


def kernel(**inputs) -> np.ndarray:
    y, _ = run(inputs, trace=False)
    return y
